# revision 4
# baseline (speedup 1.0000x reference)
"""Trainium2 Bass kernel for ConstantCurrentLIFEncode (Norse LIF encoder cell).

Reference recurrence per pixel (x = input current, constant over time):
    v_d  = v + 0.1*((0 - v) + i)        # membrane integrate
    i_d  = i + 0.2*(-i)                 # synaptic decay
    z    = (v_d - 1 > 0)                # heaviside spike
    v'   = (1 - z) * v_d                # reset on spike
    i'   = i_d + x                      # constant current inject

Algorithm (bit-identical to the f32 reference recurrence):
i_t is pixel-independent linear in x (i_t = c_t * x), so the i state is
eliminated.  With the scaled state s_t = v_t / 0.9^t the step becomes

    a   = s + g_t * x          g_t  = 0.1 * c_t / 0.9^(t+1)
    z_t = (a > th_t)           th_t = 1 / 0.9^(t+1)
    s'  = a * (a <= th_t)

On-device per step:
  * one custom DVE op (LIF_STEP_ANT): s' = select(x*g + s <= th, x*g + s, 0)
  * one spike-extract pass producing z as UINT8 (1 byte instead of 4):
      - even steps on ScalarE:  z = Exp(-1e38 * s')  -> 1.0 iff s'==0
      - odd steps on Pool/GpSimd: z = (s' == 0)      -> 1.0/0.0
    both write uint8 tiles (cast happens on the output path).
    (s'==0 <=> spike for t>=T0 given x>0; x==0 pixels are nudged to 1e-20
    on the host, which provably never spikes and keeps s'>0.)

Output compression: spikes are 0/1, so the device emits uint8 and the
host expands to f32 (pure format conversion; every bit is computed on
device).  Steps 0..T0-1 are provably all-zero for any x in [0,1) (the
x=1 upper-bound trajectory stays below threshold with huge margin), so
the host zero-fills them and the device neither computes nor stores
them.  HBM write traffic drops 4x (dtype) * 1.23x (zero prefix).

Sharding: elementwise per pixel -> flatten (c,h,w), split into 8 equal
chunks, one NeuronCore each, no communication.  Per core: [128, F] slab.
"""

import numpy as np

import concourse.bass as bass
import concourse.tile as tile
from concourse import bacc, mybir
from concourse.bass_utils import run_bass_kernel_spmd

N_CORES = 8
P = 128

F32 = mybir.dt.float32
U8 = mybir.dt.uint8


# ---------------------------------------------------------------------------
# Custom DVE op: s' = select(x*C0 + s <= C1, x*C0 + s, 0)  (one pass, 1 uop)
# ---------------------------------------------------------------------------
def _register_lif_op():
    from concourse import dve_ops
    from concourse.dve_spec import C0, C1, Spec, Src0, Src1, Zero, lower, select
    from concourse.dve_uop import DveOpSpec

    NAME = "LIF_STEP_ANT"
    if NAME in dve_ops._SUB_OPCODE_FOR_NAME:
        return next(op for op in dve_ops.OPS if op.name == NAME)

    def _ref(in0, in1, s0, s1, imm2):
        a = (in0.astype(np.float32) * np.float32(s0) + in1.astype(np.float32)).astype(
            np.float32
        )
        return np.where(a <= np.float32(s1), a, np.float32(0.0)).astype(np.float32)

    a = Src0 * C0 + Src1
    spec = Spec(body=select(a <= C1, a, Zero), reference=_ref)

    row = max(dve_ops._SUB_OPCODE_FOR_NAME.values()) + 1
    assert row < 0x20
    shas = {}
    for ver in ("v3", "v4"):
        shas[ver] = DveOpSpec(
            name=NAME, opcode=row, uops=lower(spec, ver=ver), rd1_en=True
        ).sha(ver)

    op = dve_ops.DveOp(NAME, spec, subdim=False, uops_sha=shas)
    dve_ops.OPS.append(op)
    dve_ops._SUB_OPCODE_FOR_NAME[NAME] = row
    dve_ops.CUSTOM_DVE_SPECS[NAME] = spec
    return op


_LIF_OP = _register_lif_op()


def _coefficients(steps: int):
    """Per-step accumulate gain g_t and scaled threshold th_t (f64 -> f32)."""
    g = np.zeros(steps, np.float64)
    th = np.zeros(steps, np.float64)
    c = 0.0  # i_t = c_t * x;  c_{t+1} = 0.8*c_t + 1
    for t in range(steps):
        scale = 0.9 ** (t + 1)
        g[t] = 0.1 * c / scale
        th[t] = 1.0 / scale
        c = 0.8 * c + 1.0
    return g.astype(np.float32), th.astype(np.float32), g


def _zero_prefix(steps: int) -> int:
    """Number of leading steps whose output is provably all-zero for any
    x in [0, 1): the membrane of the x=1 upper-bound trajectory (no resets
    can have happened before the first possible spike) stays below 1 with
    a margin that dwarfs f32 rounding."""
    v, c, t0 = 0.0, 0.0, 0
    for t in range(steps):
        v = 0.9 * v + 0.1 * c  # v_d at step t for x = 1
        if v >= 0.999:
            break
        t0 = t + 1
        c = 0.8 * c + 1.0
    return t0


def _build(steps: int, F: int) -> bass.Bass:
    g, th, g64 = _coefficients(steps)
    T0 = min(_zero_prefix(steps), steps - 1)
    n_live = steps - T0  # steps actually computed + stored on device

    nc = bacc.Bacc(
        "TRN2", target_bir_lowering=False, debug=False, num_devices=N_CORES
    )
    x_dram = nc.dram_tensor("x", [P, F], F32, kind="ExternalInput")
    z_dram = nc.dram_tensor("z", [n_live, P, F], U8, kind="ExternalOutput")

    with tile.TileContext(nc) as tc:
        with (
            tc.tile_pool(name="state", bufs=1) as state_pool,
            tc.tile_pool(name="upool", bufs=4) as upool,
            tc.tile_pool(name="zpool", bufs=12) as zpool,
        ):
            x = state_pool.tile([P, F], F32)
            nc.sync.dma_start(x[:], x_dram[:])

            # No spike is possible before step T0, so no resets happen and the
            # state after steps 1..T0-1 is the plain accumulation x*sum(g).
            u_prev = state_pool.tile([P, F], F32)
            if T0 > 1:
                G = float(np.float32(g64[1:T0].sum()))
                nc.vector.tensor_scalar_mul(u_prev[:], x[:], G)
            else:
                nc.vector.memset(u_prev[:], 0.0)

            for t in range(max(T0, 1), steps):
                u_new = upool.tile([P, F], F32, tag="u")
                nc.vector._custom_dve(
                    _LIF_OP,
                    out=u_new[:],
                    in0=x[:],
                    in1=u_prev[:],
                    s0=float(g[t]),
                    s1=float(th[t]),
                )
                z = zpool.tile([P, F], U8, tag="z")
                if t % 2 == 0:
                    # ScalarE: Exp(-1e38*s') == 1.0 iff s'==0 (spike), else 0.
                    nc.scalar.activation(
                        z[:], u_new[:], mybir.ActivationFunctionType.Exp, scale=-1.0e38
                    )
                else:
                    # Pool engine: z = (s' == 0)
                    nc.gpsimd.tensor_scalar(
                        z[:], u_new[:], 0.0, None, mybir.AluOpType.is_equal
                    )
                nc.sync.dma_start(z_dram[t - T0], z[:])
                u_prev = u_new

    nc.compile()
    nc._t0 = T0  # stash for kernel()
    return nc


_BUILD_CACHE: dict = {}


def kernel(input: np.ndarray, steps) -> np.ndarray:
    steps = int(steps)
    x_full = np.ascontiguousarray(np.asarray(input, dtype=np.float32))
    total = x_full.size
    assert total % (N_CORES * P) == 0, total
    F = total // (N_CORES * P)

    key = (steps, F)
    if key not in _BUILD_CACHE:
        _BUILD_CACHE[key] = _build(steps, F)
    nc = _BUILD_CACHE[key]
    T0 = nc._t0

    x_flat = x_full.reshape(N_CORES, P, F)
    # x == 0 pixels never spike; nudge to 1e-20 (also never spikes, by a
    # ~1e19x margin) so "state == 0" remains equivalent to "spiked".
    x_flat = np.where(x_flat == 0.0, np.float32(1e-20), x_flat)
    in_maps = [{"x": x_flat[c]} for c in range(N_CORES)]
    res = run_bass_kernel_spmd(nc, in_maps, list(range(N_CORES)))

    out = np.zeros((steps, N_CORES, P * F), np.float32)
    for c in range(N_CORES):
        zc = res.results[c]["z"].reshape(steps - T0, P * F)
        out[T0:, c, :] = zc  # uint8 -> float32 expand (0.0 / 1.0)
    return out.reshape((steps,) + x_full.shape)


# revision 5
# speedup vs baseline: 4.8245x; 4.8245x over previous
"""Trainium2 Bass kernel for ConstantCurrentLIFEncode (Norse LIF encoder cell).

Reference recurrence per pixel (x = input current, constant over time):
    v_d  = v + 0.1*((0 - v) + i)        # membrane integrate
    i_d  = i + 0.2*(-i)                 # synaptic decay
    z    = (v_d - 1 > 0)                # heaviside spike
    v'   = (1 - z) * v_d                # reset on spike
    i'   = i_d + x                      # constant current inject

Algorithm (bit-identical to the f32 reference recurrence):
i_t is pixel-independent linear in x (i_t = c_t * x), so the i state is
eliminated.  With the scaled state s_t = v_t / 0.9^t the step becomes

    a   = s + g_t * x          g_t  = 0.1 * c_t / 0.9^(t+1)
    z_t = (a > th_t)           th_t = 1 / 0.9^(t+1)
    s'  = a * (a <= th_t)

On-device per step:
  * one custom DVE op (LIF_STEP_ANT): s' = select(x*g + s <= th, x*g + s, 0)
  * one spike-extract pass producing z as UINT8 (1 byte instead of 4):
      - even steps on ScalarE:  z = Exp(-1e38 * s')  -> 1.0 iff s'==0
      - odd steps on Pool/GpSimd: z = (s' == 0)      -> 1.0/0.0
    both write uint8 tiles (cast happens on the output path).
    (s'==0 <=> spike for t>=T0 given x>0; x==0 pixels are nudged to 1e-20
    on the host, which provably never spikes and keeps s'>0.)

Output compression: spikes are 0/1, so the device emits uint8 and the
host expands to f32 (pure format conversion; every bit is computed on
device).  Steps 0..T0-1 are provably all-zero for any x in [0,1) (the
x=1 upper-bound trajectory stays below threshold with huge margin), so
the host zero-fills them and the device neither computes nor stores
them.  HBM write traffic drops 4x (dtype) * 1.23x (zero prefix).

Sharding: elementwise per pixel -> flatten (c,h,w), split into 8 equal
chunks, one NeuronCore each, no communication.  Per core: [128, F] slab.
"""

import numpy as np

import concourse.bass as bass
import concourse.tile as tile
from concourse import bacc, mybir
from concourse.bass_utils import run_bass_kernel_spmd

N_CORES = 8
P = 128

F32 = mybir.dt.float32
U8 = mybir.dt.uint8


# ---------------------------------------------------------------------------
# Custom DVE op: s' = select(x*C0 + s <= C1, x*C0 + s, 0)  (one pass, 1 uop)
# ---------------------------------------------------------------------------
def _register_lif_op():
    from concourse import dve_ops
    from concourse.dve_spec import C0, C1, Spec, Src0, Src1, Zero, lower, select
    from concourse.dve_uop import DveOpSpec

    NAME = "LIF_STEP_ANT"
    if NAME in dve_ops._SUB_OPCODE_FOR_NAME:
        return next(op for op in dve_ops.OPS if op.name == NAME)

    def _ref(in0, in1, s0, s1, imm2):
        a = (in0.astype(np.float32) * np.float32(s0) + in1.astype(np.float32)).astype(
            np.float32
        )
        return np.where(a <= np.float32(s1), a, np.float32(0.0)).astype(np.float32)

    a = Src0 * C0 + Src1
    spec = Spec(body=select(a <= C1, a, Zero), reference=_ref)

    row = max(dve_ops._SUB_OPCODE_FOR_NAME.values()) + 1
    assert row < 0x20
    shas = {}
    for ver in ("v3", "v4"):
        shas[ver] = DveOpSpec(
            name=NAME, opcode=row, uops=lower(spec, ver=ver), rd1_en=True
        ).sha(ver)

    op = dve_ops.DveOp(NAME, spec, subdim=False, uops_sha=shas)
    dve_ops.OPS.append(op)
    dve_ops._SUB_OPCODE_FOR_NAME[NAME] = row
    dve_ops.CUSTOM_DVE_SPECS[NAME] = spec
    return op


_LIF_OP = _register_lif_op()


def _coefficients(steps: int):
    """Per-step accumulate gain g_t and scaled threshold th_t (f64 -> f32)."""
    g = np.zeros(steps, np.float64)
    th = np.zeros(steps, np.float64)
    c = 0.0  # i_t = c_t * x;  c_{t+1} = 0.8*c_t + 1
    for t in range(steps):
        scale = 0.9 ** (t + 1)
        g[t] = 0.1 * c / scale
        th[t] = 1.0 / scale
        c = 0.8 * c + 1.0
    return g.astype(np.float32), th.astype(np.float32), g


def _zero_prefix(steps: int) -> int:
    """Number of leading steps whose output is provably all-zero for any
    x in [0, 1): the membrane of the x=1 upper-bound trajectory (no resets
    can have happened before the first possible spike) stays below 1 with
    a margin that dwarfs f32 rounding."""
    v, c, t0 = 0.0, 0.0, 0
    for t in range(steps):
        v = 0.9 * v + 0.1 * c  # v_d at step t for x = 1
        if v >= 0.999:
            break
        t0 = t + 1
        c = 0.8 * c + 1.0
    return t0


def _build(steps: int, F: int) -> bass.Bass:
    g, th, g64 = _coefficients(steps)
    T0 = min(_zero_prefix(steps), steps - 1)
    n_live = steps - T0  # steps actually computed + stored on device

    nc = bacc.Bacc(
        "TRN2", target_bir_lowering=False, debug=False, num_devices=N_CORES
    )
    x_dram = nc.dram_tensor("x", [P, F], F32, kind="ExternalInput")
    z_dram = nc.dram_tensor("z", [n_live, P, F], U8, kind="ExternalOutput")

    with tile.TileContext(nc) as tc:
        with (
            tc.tile_pool(name="state", bufs=1) as state_pool,
            tc.tile_pool(name="upool", bufs=4) as upool,
            tc.tile_pool(name="zpool", bufs=12) as zpool,
        ):
            x = state_pool.tile([P, F], F32)
            nc.sync.dma_start(x[:], x_dram[:])

            # No spike is possible before step T0, so no resets happen and the
            # state after steps 1..T0-1 is the plain accumulation x*sum(g).
            u_prev = state_pool.tile([P, F], F32)
            if T0 > 1:
                G = float(np.float32(g64[1:T0].sum()))
                nc.vector.tensor_scalar_mul(u_prev[:], x[:], G)
            else:
                nc.vector.memset(u_prev[:], 0.0)

            for t in range(max(T0, 1), steps):
                u_new = upool.tile([P, F], F32, tag="u")
                nc.vector._custom_dve(
                    _LIF_OP,
                    out=u_new[:],
                    in0=x[:],
                    in1=u_prev[:],
                    s0=float(g[t]),
                    s1=float(th[t]),
                )
                z = zpool.tile([P, F], U8, tag="z")
                if t % 3 == 2:
                    # VectorE (DVE): z = (s' == 0); ~0.65us vs 1.3us on ScalarE.
                    nc.vector.tensor_scalar(
                        z[:], u_new[:], 0.0, None, mybir.AluOpType.is_equal
                    )
                else:
                    # ScalarE: Exp(-1e38*s') == 1.0 iff s'==0 (spike), else 0.
                    nc.scalar.activation(
                        z[:], u_new[:], mybir.ActivationFunctionType.Exp, scale=-1.0e38
                    )
                nc.sync.dma_start(z_dram[t - T0], z[:])
                u_prev = u_new

    nc.compile()
    nc._t0 = T0  # stash for kernel()
    return nc


_BUILD_CACHE: dict = {}


def kernel(input: np.ndarray, steps) -> np.ndarray:
    steps = int(steps)
    x_full = np.ascontiguousarray(np.asarray(input, dtype=np.float32))
    total = x_full.size
    assert total % (N_CORES * P) == 0, total
    F = total // (N_CORES * P)

    key = (steps, F)
    if key not in _BUILD_CACHE:
        _BUILD_CACHE[key] = _build(steps, F)
    nc = _BUILD_CACHE[key]
    T0 = nc._t0

    x_flat = x_full.reshape(N_CORES, P, F)
    # x == 0 pixels never spike; nudge to 1e-20 (also never spikes, by a
    # ~1e19x margin) so "state == 0" remains equivalent to "spiked".
    x_flat = np.where(x_flat == 0.0, np.float32(1e-20), x_flat)
    in_maps = [{"x": x_flat[c]} for c in range(N_CORES)]
    res = run_bass_kernel_spmd(nc, in_maps, list(range(N_CORES)))

    out = np.zeros((steps, N_CORES, P * F), np.float32)
    for c in range(N_CORES):
        zc = res.results[c]["z"].reshape(steps - T0, P * F)
        out[T0:, c, :] = zc  # uint8 -> float32 expand (0.0 / 1.0)
    return out.reshape((steps,) + x_full.shape)


# revision 6
# speedup vs baseline: 5.0453x; 1.0458x over previous
"""Trainium2 Bass kernel for ConstantCurrentLIFEncode — fused triple-step DVE.

Scaled-state LIF recurrence (see kernel.py docstring):
    a_t  = s + g_t * x ;  z_t = (a_t > th_t) ;  s' = a_t * (a_t <= th_t)

The DVE custom-op pipeline has 8 ALU stages and processes 1 elem/cycle
regardless of body depth, so deeper fused bodies are FREE.  The 3 scalar
const slots limit fusion to 1.5 steps/op:

  PAIR_A(x, s)  -> aB   (step t fully + step t+1 accumulate, 6 stages)
      mA=x*G_A; aA=s+mA; cA=[aA<=T_A]; rA=sel(cA,aA,0); mB=mA*R; aB=rA+mB
  PAIR_B(x, aB) -> rC   (step t+1 mask + step t+2 fully, 6 stages)
      cB=[aB<=T_B]; rB=sel(cB,aB,0); mC=x*G_C; aC=rB+mC; cC=[aC<=T_C];
      rC=sel(cC,aC,0)
  EXT_AB(x, aB) -> zA + 2*zB as uint8  (7 stages)
      mA=x*G_A; mB=mA*R  (bit-identical replica of PAIR_A's mB);
      zA=[aB-mB==0]  (aB==mB <=> rA==0 <=> spiked at t);
      zB=[aB>T_B]; out=zA+2*zB
  zC plane: ScalarE Exp(-1e38*rC) -> uint8 (1 iff rC==0 <=> spiked at t+2)

Per 3 steps: 3 Vector ops + 1 Scalar op + 2 uint8 planes of DMA.
Steps 0..T0-1 are provably all-zero -> host fills.  Leftover (26%3=2)
steps use the single-step op + ScalarE Exp.

zA robustness: rA>0 => aB=rA+mB with rA/mB >= G_A/G_B ~ 0.85, so f32
never absorbs rA into mB; rA==0 => aB==mB exactly (0+mB).  x==0 pixels
nudged to 1e-20 on host (never spike, keep aA>0).
"""

import numpy as np

import concourse.bass as bass
import concourse.tile as tile
from concourse import bacc, mybir
from concourse.bass_utils import run_bass_kernel_spmd

N_CORES = 8
P = 128

F32 = mybir.dt.float32
U8 = mybir.dt.uint8


def _register_ops():
    from concourse import dve_ops
    from concourse.dve_spec import (
        C0,
        C1,
        C2,
        Spec,
        Src0,
        Src1,
        Zero,
        eq,
        lower,
        select,
    )
    from concourse.dve_uop import DveOpSpec

    def _mk(name, spec):
        if name in dve_ops._SUB_OPCODE_FOR_NAME:
            return next(op for op in dve_ops.OPS if op.name == name)
        row = max(dve_ops._SUB_OPCODE_FOR_NAME.values()) + 1
        assert row < 0x20
        shas = {}
        for ver in ("v3", "v4"):
            shas[ver] = DveOpSpec(
                name=name, opcode=row, uops=lower(spec, ver=ver), rd1_en=True
            ).sha(ver)
        op = dve_ops.DveOp(name, spec, subdim=False, uops_sha=shas)
        dve_ops.OPS.append(op)
        dve_ops._SUB_OPCODE_FOR_NAME[name] = row
        dve_ops.CUSTOM_DVE_SPECS[name] = spec
        return op

    f32 = np.float32

    # --- single step: s' = select(x*C0 + s <= C1, x*C0 + s, 0)
    def _ref_step(in0, in1, s0, s1, imm2):
        a = (in0.astype(f32) * f32(s0) + in1.astype(f32)).astype(f32)
        return np.where(a <= f32(s1), a, f32(0.0)).astype(f32)

    a = Src0 * C0 + Src1
    step = _mk("LIF_STEP_ANT", Spec(body=select(a <= C1, a, Zero), reference=_ref_step))

    # --- PAIR_A: C0=G_A, C1=T_A, C2=R(=G_B/G_A as f32)
    def _ref_pa(in0, in1, s0, s1, imm2):
        x = in0.astype(f32)
        mA = (x * f32(s0)).astype(f32)
        aA = (in1.astype(f32) + mA).astype(f32)
        rA = np.where(aA <= f32(s1), aA, f32(0.0)).astype(f32)
        mB = (mA * f32(imm2)).astype(f32)
        return (rA + mB).astype(f32)

    mA = Src0 * C0
    aA = mA + Src1
    rA = select(aA <= C1, aA, Zero)
    pair_a = _mk("LIF_PAIR_A_ANT", Spec(body=rA + mA * C2, reference=_ref_pa))

    # --- PAIR_B: C0=T_B, C1=G_C, C2=T_C
    def _ref_pb(in0, in1, s0, s1, imm2):
        aB = in1.astype(f32)
        rB = np.where(aB <= f32(s0), aB, f32(0.0)).astype(f32)
        mC = (in0.astype(f32) * f32(s1)).astype(f32)
        aC = (rB + mC).astype(f32)
        return np.where(aC <= f32(imm2), aC, f32(0.0)).astype(f32)

    rB = select(Src1 <= C0, Src1, Zero)
    aC = rB + Src0 * C1
    pair_b = _mk("LIF_PAIR_B_ANT", Spec(body=select(aC <= C2, aC, Zero), reference=_ref_pb))

    # --- EXT_AB: C0=G_A, C1=R, C2=T_B ; out = zA + 2*zB (uint8)
    def _ref_ext(in0, in1, s0, s1, imm2):
        x = in0.astype(f32)
        aB = in1.astype(f32)
        mA = (x * f32(s0)).astype(f32)
        mB = (mA * f32(s1)).astype(f32)
        d = (aB - mB).astype(f32)
        zA = (d == f32(0.0)).astype(f32)
        zB = (aB > f32(imm2)).astype(f32)
        return (zA + zB + zB).astype(f32)

    mA2 = Src0 * C0
    mB2 = mA2 * C1
    zA = eq(Src1 - mB2, Zero)
    zB = Src1 > C2
    ext = _mk("LIF_EXT_AB_ANT", Spec(body=zA + (zB + zB), reference=_ref_ext))

    # --- PAIR_A_FIRST (1-src): state before first triple is x*Gpre; fold it.
    # C0=Gtot(=Gpre+G_A), C1=T_A, C2=G_B.  out = rA + x*G_B
    def _ref_paf(in0, in1, s0, s1, imm2):
        x = in0.astype(f32)
        aA = (x * f32(s0)).astype(f32)
        rA = np.where(aA <= f32(s1), aA, f32(0.0)).astype(f32)
        mB = (x * f32(imm2)).astype(f32)
        return (rA + mB).astype(f32)

    aAf = Src0 * C0
    rAf = select(aAf <= C1, aAf, Zero)
    pair_af = _mk("LIF_PAIR_AF_ANT", Spec(body=rAf + Src0 * C2, reference=_ref_paf))

    # --- EXT_FIRST: C0=G_B, C1=T_B ; mB=x*G_B bit-identical to PAIR_AF's.
    def _ref_extf(in0, in1, s0, s1, imm2):
        x = in0.astype(f32)
        aB = in1.astype(f32)
        mB = (x * f32(s0)).astype(f32)
        zA = ((aB - mB).astype(f32) == f32(0.0)).astype(f32)
        zB = (aB > f32(s1)).astype(f32)
        return (zA + zB + zB).astype(f32)

    mBf = Src0 * C0
    zAf = eq(Src1 - mBf, Zero)
    zBf = Src1 > C1
    ext_f = _mk("LIF_EXT_F_ANT", Spec(body=zAf + (zBf + zBf), reference=_ref_extf))

    return step, pair_a, pair_b, ext, pair_af, ext_f


_STEP_OP, _PAIR_A, _PAIR_B, _EXT, _PAIR_AF, _EXT_F = _register_ops()


def _coefficients(steps: int):
    g = np.zeros(steps, np.float64)
    th = np.zeros(steps, np.float64)
    c = 0.0
    for t in range(steps):
        scale = 0.9 ** (t + 1)
        g[t] = 0.1 * c / scale
        th[t] = 1.0 / scale
        c = 0.8 * c + 1.0
    return g.astype(np.float32), th.astype(np.float32), g


def _zero_prefix(steps: int) -> int:
    v, c, t0 = 0.0, 0.0, 0
    for t in range(steps):
        v = 0.9 * v + 0.1 * c
        if v >= 0.999:
            break
        t0 = t + 1
        c = 0.8 * c + 1.0
    return t0


def _plan(steps: int):
    """Returns (T0, triples, singles): triples start at t, cover t..t+2."""
    T0 = min(_zero_prefix(steps), steps - 1)
    live = steps - max(T0, 1)
    first = max(T0, 1)
    n3 = live // 3
    triples = [first + 3 * k for k in range(n3)]
    singles = list(range(first + 3 * n3, steps))
    return T0, triples, singles


def _build(steps: int, F: int) -> bass.Bass:
    g, th, g64 = _coefficients(steps)
    T0, triples, singles = _plan(steps)
    n_planes = 2 * len(triples) + len(singles)

    nc = bacc.Bacc(
        "TRN2", target_bir_lowering=False, debug=False, num_devices=N_CORES
    )
    x_dram = nc.dram_tensor("x", [P, F], F32, kind="ExternalInput")
    z_dram = nc.dram_tensor("z", [n_planes, P, F], U8, kind="ExternalOutput")

    with tile.TileContext(nc) as tc:
        with (
            tc.tile_pool(name="state", bufs=1) as state_pool,
            tc.tile_pool(name="upool", bufs=6) as upool,
            tc.tile_pool(name="zpool", bufs=12) as zpool,
        ):
            x = state_pool.tile([P, F], F32)
            # split the load across both HWDGE issue queues (gpsimd SWDGE
            # costs a 2.5us drain at block exit -- not worth it)
            nc.sync.dma_start(x[0:64, :], x_dram[0:64, :])
            nc.scalar.dma_start(x[64:128, :], x_dram[64:128, :])

            u_prev = None
            if not (T0 > 1 and triples and triples[0] == max(T0, 1)):
                u_prev = state_pool.tile([P, F], F32)
                if T0 > 1:
                    G = float(np.float32(g64[1:T0].sum()))
                    nc.scalar.mul(u_prev[:], x[:], G)
                else:
                    nc.vector.memset(u_prev[:], 0.0)

            plane = 0
            for t in triples:
                ratio = float(np.float32(g64[t + 1] / g64[t]))
                aB = upool.tile([P, F], F32, tag="u")
                if u_prev is None:
                    # first triple: state = x*Gpre folds into a 1-src op
                    Gtot = float(np.float32(g64[1:T0].sum() + g64[t]))
                    gB = float(g[t + 1])
                    nc.vector._custom_dve(
                        _PAIR_AF,
                        out=aB[:], in0=x[:],
                        s0=Gtot, s1=float(th[t]), imm2=gB,
                    )
                    ex = zpool.tile([P, F], U8, tag="z")
                    nc.vector._custom_dve(
                        _EXT_F,
                        out=ex[:], in0=x[:], in1=aB[:],
                        s0=gB, s1=float(th[t + 1]),
                    )
                else:
                    nc.vector._custom_dve(
                        _PAIR_A,
                        out=aB[:], in0=x[:], in1=u_prev[:],
                        s0=float(g[t]), s1=float(th[t]), imm2=ratio,
                    )
                    ex = zpool.tile([P, F], U8, tag="z")
                    nc.vector._custom_dve(
                        _EXT,
                        out=ex[:], in0=x[:], in1=aB[:],
                        s0=float(g[t]), s1=ratio, imm2=float(th[t + 1]),
                    )
                nc.sync.dma_start(z_dram[plane], ex[:])  # noqa: ex-plane on sync queue
                u_new = upool.tile([P, F], F32, tag="u")
                nc.vector._custom_dve(
                    _PAIR_B,
                    out=u_new[:], in0=x[:], in1=aB[:],
                    s0=float(th[t + 1]), s1=float(g[t + 2]), imm2=float(th[t + 2]),
                )
                zc = zpool.tile([P, F], U8, tag="z")
                nc.scalar.activation(
                    zc[:], u_new[:], mybir.ActivationFunctionType.Exp, scale=-1.0e38
                )
                nc.sync.dma_start(z_dram[plane + 1], zc[:])
                plane += 2
                u_prev = u_new

            for t in singles:
                u_new = upool.tile([P, F], F32, tag="u")
                nc.vector._custom_dve(
                    _STEP_OP,
                    out=u_new[:], in0=x[:], in1=u_prev[:],
                    s0=float(g[t]), s1=float(th[t]),
                )
                z = zpool.tile([P, F], U8, tag="z")
                nc.scalar.activation(
                    z[:], u_new[:], mybir.ActivationFunctionType.Exp, scale=-1.0e38
                )
                nc.sync.dma_start(z_dram[plane], z[:])
                plane += 1
                u_prev = u_new

    nc.compile()
    nc._plan = (T0, triples, singles)
    return nc


_BUILD_CACHE: dict = {}


def kernel(input: np.ndarray, steps) -> np.ndarray:
    steps = int(steps)
    x_full = np.ascontiguousarray(np.asarray(input, dtype=np.float32))
    total = x_full.size
    assert total % (N_CORES * P) == 0, total
    F = total // (N_CORES * P)

    key = (steps, F)
    if key not in _BUILD_CACHE:
        _BUILD_CACHE[key] = _build(steps, F)
    nc = _BUILD_CACHE[key]
    T0, triples, singles = nc._plan

    x_flat = x_full.reshape(N_CORES, P, F)
    x_flat = np.where(x_flat == 0.0, np.float32(1e-20), x_flat)
    in_maps = [{"x": x_flat[c]} for c in range(N_CORES)]
    res = run_bass_kernel_spmd(nc, in_maps, list(range(N_CORES)))

    out = np.zeros((steps, N_CORES, P * F), np.float32)
    for c in range(N_CORES):
        zc_all = res.results[c]["z"].reshape(-1, P * F)
        plane = 0
        for t in triples:
            ex = zc_all[plane]
            out[t, c, :] = (ex & 1).astype(np.float32)
            out[t + 1, c, :] = ((ex >> 1) & 1).astype(np.float32)
            out[t + 2, c, :] = zc_all[plane + 1].astype(np.float32)
            plane += 2
        for t in singles:
            out[t, c, :] = zc_all[plane].astype(np.float32)
            plane += 1
    return out.reshape((steps,) + x_full.shape)
